# revision 10
# baseline (speedup 1.0000x reference)
"""Trainium2 Bass kernel for the NeRF coordinate-chain problem.

Reference semantics: flat_dihedrals [1048576, 3] is (row-major) reinterpreted
as phi[K=6144, B=512]; each of the 512 columns is an independent serial NeRF
chain of K rigid-body extension steps, with bond-geometry constants cycling as
d = (q*B + b) mod 3.

Key reformulation: the per-step update is an affine (SE3) composition
    T_q = T_{q-1} o A_q,   A_q = [[G(phi,theta_d), p],[0,1]],
    G = Rx(phi) @ Rz(theta_d),  p = r_d * G[:,0],   T_0 = Identity
and coord_q = translation(T_q).  Associativity turns the 6144-step serial
recurrence into a blocked parallel scan:
  L0 upsweep: 256 blocks of S0=24 steps, vectorized over (block x batch)
    chains (structured Rx/Rz composes; w = r_d * c0_new);
  block-prefix: R-only Brent-Kung scan over the 128 blocks of each
    partition-half + translations via batched matvec and the hardware
    tensor_tensor_scan cumsum; cross-half fixup via a tiny SBUF DMA;
  apply: coords = ShR @ W + ShT per atom, chunked and pipelined with
    PE-transposes into k-major layout and contiguous output DMAs
    (GpSimd carries the contiguous adds).

Sharding: batch columns are split across 8 cores (64 columns/core); the
per-core chain layout uses partitions p = c + 64*nb2, free dims (q, s) with
block id nb = 128*nb2 + q.
"""

import numpy as np

L_STEPS = 2048
B_FULL = 512
NUM_CORES = 8
BC = B_FULL // NUM_CORES          # batch columns per core
K = 3 * L_STEPS                   # chain length = 6144
S0 = 24                           # L0 block size (multiple of 3)
NQ = 128                          # blocks per partition-half
NB2 = 2                           # partition halves (nb = 128*nb2 + q)
NB0 = NQ * NB2                    # 256 L0 blocks

_BL = np.array([145.801, 152.326, 132.868], dtype=np.float32)
_BA = np.array([2.124, 1.941, 2.028], dtype=np.float32)
_CT = np.cos(np.pi - _BA).astype(np.float32)
_ST = np.sin(np.pi - _BA).astype(np.float32)
_RCT = (_BL * _CT).astype(np.float32)
_RST = (_BL * _ST).astype(np.float32)

_CACHE = {}


def _build_program(reps: int = 1, only: str = ""):
    """Build the program.  `only` repeats a single phase inside the reps loop
    ("l0" | "scan" | "apply") for phase-level HW timing; "" = full kernel
    repeated per rep."""
    import concourse.bass as bass
    import concourse.tile as tile
    from concourse import bacc, masks, mybir
    from concourse._compat import axon_active

    f32 = mybir.dt.float32
    Al = mybir.AluOpType
    Act = mybir.ActivationFunctionType

    nc = bacc.Bacc(
        "TRN2",
        target_bir_lowering=False,
        debug=not axon_active(),
        enable_asserts=False,
        num_devices=NUM_CORES,
    )
    phi_d = nc.dram_tensor("phi", [3, 128, NQ, S0 // 3], f32, kind="ExternalInput").ap()
    mtab_d = nc.dram_tensor("mtab", [128, S0, 4], f32, kind="ExternalInput").ap()
    out_d = nc.dram_tensor("out", [K, BC, 3], f32, kind="ExternalOutput").ap()

    with tile.TileContext(nc) as tc:
        with tc.tile_pool(name="main", bufs=1) as pool:
            S = {}

            def ph_l0():
                # ---------------- load inputs + trig ----------------
                mtab = S["mtab"] = pool.tile([128, S0, 4], f32, tag="mtab", name="mtab")
                nc.sync.dma_start(mtab[:], mtab_d[:])
                SC = S0 // 3  # s-chunk size
                pih = pool.tile([128, 1], f32, tag="pih", name="pih")
                zero = pool.tile([128, 1], f32, tag="zero", name="zero")
                nc.vector.memset(pih[:], float(np.pi / 2))
                nc.vector.memset(zero[:], 0.0)
                # sc[..., 0]=cos(phi), 1=sin(phi), 2=-sin(phi); chunked over s
                scs = []
                for cch in range(3):
                    phi = pool.tile([128, NQ, SC], f32, tag=f"phi{cch}", name=f"phi{cch}")
                    nc.sync.dma_start(phi[:], phi_d[cch])
                    sct = pool.tile([128, NQ, SC, 3], f32, tag=f"sc{cch}", name=f"sc{cch}")
                    absphi = pool.tile([128, NQ, SC], f32, tag=f"abs{cch}", name=f"abs{cch}")
                    nc.scalar.activation(absphi[:], phi[:], Act.Abs, bias=zero[:, :])
                    nc.scalar.activation(sct[:, :, :, 1], phi[:], Act.Sin, bias=zero[:, :])
                    nc.scalar.activation(sct[:, :, :, 0], absphi[:], Act.Sin, bias=pih[:, :], scale=-1.0)
                    nc.scalar.activation(sct[:, :, :, 2], phi[:], Act.Sin, bias=zero[:, :], scale=-1.0)
                    scs.append(sct)

                # ---------------- L0 upsweep ----------------
                # R state [c0|c1|c2] column-major, per (p, q) chain
                R = S["R"] = pool.tile([128, NQ, 9], f32, tag="R", name="R")
                nc.vector.memset(R[:], 0.0)
                nc.vector.memset(R[:, :, 0:9:4], 1.0)
                W = S["W"] = pool.tile([128, NQ, S0, 3], f32, tag="W", name="W")
                m12 = pool.tile([128, NQ, 2, 3], f32, tag="m12", name="m12")
                m34 = pool.tile([128, NQ, 2, 3], f32, tag="m34", name="m34")
                tb0 = pool.tile([128, NQ, 3], f32, tag="tb0", name="tb0")
                tb1 = pool.tile([128, NQ, 3], f32, tag="tb1", name="tb1")

                R12 = R[:, :, 3:9].rearrange("p q (two three) -> p q two three", two=2)
                R21 = R12[:, :, ::-1, :]

                for s in range(S0):
                    sct = scs[s // SC]
                    cphB = sct[:, :, s % SC, 0:1].unsqueeze(2).broadcast_to([128, NQ, 2, 3])
                    snsB = sct[:, :, s % SC, 1:3].unsqueeze(3).broadcast_to([128, NQ, 2, 3])
                    # Rx(phi): (c1,c2) <- (cp*c1+sp*c2, cp*c2-sp*c1)
                    nc.vector.tensor_tensor(m12[:], R12, cphB, op=Al.mult)
                    nc.vector.tensor_tensor(m34[:], R21, snsB, op=Al.mult)
                    nc.vector.tensor_tensor(R12, m12[:], m34[:], op=Al.add)
                    # Rz(theta): (c0,c1) <- (ct*c0+st*c1, ct*c1-st*c0), fused stt
                    nc.vector.tensor_scalar(tb0[:], R[:, :, 3:6], mtab[:, s, 1:2], None, op0=Al.mult)
                    nc.vector.tensor_scalar(tb1[:], R[:, :, 0:3], mtab[:, s, 1:2], None, op0=Al.mult)
                    nc.vector.scalar_tensor_tensor(
                        R[:, :, 0:3], R[:, :, 0:3], mtab[:, s, 0:1], tb0[:], op0=Al.mult, op1=Al.add
                    )
                    nc.vector.scalar_tensor_tensor(
                        R[:, :, 3:6], R[:, :, 3:6], mtab[:, s, 0:1], tb1[:], op0=Al.mult, op1=Al.subtract
                    )
                    # w = R_old@p = r_d * c0_new ; W[s] = W[s-1] + w   (fused stt)
                    if s == 0:
                        nc.vector.tensor_scalar(W[:, :, 0, :], R[:, :, 0:3], mtab[:, s, 2:3], None, op0=Al.mult)
                    else:
                        nc.vector.scalar_tensor_tensor(
                            W[:, :, s, :], R[:, :, 0:3], mtab[:, s, 2:3], W[:, :, s - 1, :],
                            op0=Al.mult, op1=Al.add,
                        )

            def ph_scan():
                # ================= block-prefix phase =================
                R, W = S["R"], S["W"]
                ma = pool.tile([128, NQ, 9], f32, tag="sc0", name="ma")
                mb = pool.tile([128, NQ, 9], f32, tag="sc1", name="mb")
                mc = pool.tile([128, NQ, 9], f32, tag="sc2", name="mc")

                def compose_R(dst, a_R, b_R, P, g):
                    """dst[9] = Ra @ Rb (column-major cols)."""
                    pb = dst.base_partition()
                    for kk in range(3):
                        colk = a_R[:, :, 3 * kk : 3 * kk + 3].unsqueeze(2).broadcast_to([P, g, 3, 3])
                        rowk = b_R[:, :, kk::3].unsqueeze(3).broadcast_to([P, g, 3, 3])
                        dst_m = (ma, mb, mc)[kk]
                        mv = dst_m[pb : pb + P, 0:g].rearrange("p g (f t) -> p g f t", f=3)
                        nc.vector.tensor_tensor(mv, colk, rowk, op=Al.mult)
                    nc.vector.tensor_tensor(ma[pb : pb + P, 0:g], ma[pb : pb + P, 0:g], mb[pb : pb + P, 0:g], op=Al.add)
                    nc.vector.tensor_tensor(dst, ma[pb : pb + P, 0:g], mc[pb : pb + P, 0:g], op=Al.add)

                # Brent-Kung in-place inclusive scan over the q axis (per half)
                d = 1
                while d < NQ:
                    n = NQ // (2 * d)
                    a = R[:].rearrange("p (m j) t -> p m j t", j=2 * d)[:, :, d - 1, :]
                    b = R[:].rearrange("p (m j) t -> p m j t", j=2 * d)[:, :, 2 * d - 1, :]
                    compose_R(b, a, b, 128, n)
                    d *= 2
                d = NQ // 4
                while d >= 1:
                    n = NQ // (2 * d) - 1
                    a = R[:].rearrange("p (m j) t -> p m j t", j=2 * d)[:, 0:n, 2 * d - 1, :]
                    b = R[:].rearrange("p (m j) t -> p m j t", j=2 * d)[:, 1 : n + 1, d - 1, :]
                    compose_R(b, a, b, 128, n)
                    d //= 2
                PR = R  # inclusive R-prefix per half, in place

                # local shifted prefix (identity at q=0, both halves)
                ShR = S["ShR"] = pool.tile([128, NQ, 9], f32, tag="ShR", name="ShR")
                nc.vector.tensor_copy(ShR[:, 1:NQ, :], PR[:, 0 : NQ - 1, :])
                nc.vector.memset(ShR[:, 0, :], 0.0)
                nc.vector.memset(ShR[:, 0, 0:9:4], 1.0)

                # v[q] = ShR_local[q] @ t_loc[q] ; TI = cumsum_q(v) per half
                tloc = W[:, :, S0 - 1, :]
                v = pool.tile([128, NQ, 3], f32, tag="v", name="v")
                vz = pool.tile([128, NQ], f32, tag="vz", name="vz")
                nc.vector.memset(vz[:], 0.0)
                for kk in range(3):
                    colk = ShR[:, :, 3 * kk : 3 * kk + 3]
                    tk = tloc[:, :, kk : kk + 1].broadcast_to([128, NQ, 3])
                    dst_m = (ma, mb, mc)[kk]
                    nc.vector.tensor_tensor(dst_m[:, :, 0:3], colk, tk, op=Al.mult)
                nc.vector.tensor_tensor(ma[:, :, 0:3], ma[:, :, 0:3], mb[:, :, 0:3], op=Al.add)
                nc.vector.tensor_tensor(v[:], ma[:, :, 0:3], mc[:, :, 0:3], op=Al.add)
                TI = pool.tile([128, NQ, 3], f32, tag="TI", name="TI")
                for i in range(3):
                    nc.vector.tensor_tensor_scan(
                        TI[:, :, i], v[:, :, i], vz[:], 0.0, op0=Al.add, op1=Al.add
                    )
                # shifted translation prefix
                ShT = S["ShT"] = pool.tile([128, NQ, 3], f32, tag="ShT", name="ShT")
                nc.vector.tensor_copy(ShT[:, 1:NQ, :], TI[:, 0 : NQ - 1, :])
                nc.vector.memset(ShT[:, 0, :], 0.0)

                # cross-half: bring lower-half totals to upper partitions
                stgR = pool.tile([128, 1, 9], f32, tag="stgR", name="stgR")
                stgT = pool.tile([128, 1, 3], f32, tag="stgT", name="stgT")
                nc.sync.dma_start(stgR[64:128, :, :], PR[0:64, NQ - 1 : NQ, :])
                nc.sync.dma_start(stgT[64:128, :, :], TI[0:64, NQ - 1 : NQ, :])
                # ShR_up <- R_lowtot o ShR_up (in place)
                aR = stgR[64:128, :, :].broadcast_to([64, NQ, 9])
                compose_R(ShR[64:128, :, :], aR, ShR[64:128, :, :], 64, NQ)
                # ShT_up <- R_lowtot @ ShT_up + t_lowtot
                for kk in range(3):
                    colk = stgR[64:128, :, 3 * kk : 3 * kk + 3].broadcast_to([64, NQ, 3])
                    tk = ShT[64:128, :, kk : kk + 1].broadcast_to([64, NQ, 3])
                    dst_m = (ma, mb, mc)[kk]
                    nc.vector.tensor_tensor(dst_m[64:128, :, 0:3], colk, tk, op=Al.mult)
                nc.vector.tensor_tensor(ma[64:128, :, 0:3], ma[64:128, :, 0:3], mb[64:128, :, 0:3], op=Al.add)
                nc.vector.tensor_tensor(ShT[64:128, :, :], ma[64:128, :, 0:3], mc[64:128, :, 0:3], op=Al.add)
                nc.vector.tensor_tensor(
                    ShT[64:128, :, :], ShT[64:128, :, :],
                    stgT[64:128, :, :].broadcast_to([64, NQ, 3]), op=Al.add,
                )

            def ph_apply():
                # -------- L0 apply + transpose + store, pipelined by q-chunks --------
                # Each (chunk, i) unit runs entirely on one engine (2 on DVE, 1
                # on Pool) with disjoint scratch, so the engines proceed
                # concurrently with no per-step cross-engine ping-pong.
                W, ShR, ShT = S["W"], S["ShR"], S["ShT"]
                ident = pool.tile([128, 128], f32, tag="ident", name="ident")
                masks.make_identity(nc, ident[:])
                out_dv = out_d.rearrange("(kk p) c i -> p kk c i", p=128)
                NKB = 3072 // 128          # 24 kb tiles per half
                QC = 32                    # q-chunk; 32*24 = 768 = 6 kb tiles
                NCH = NQ // QC
                u0 = pool.tile([128, QC, S0], f32, tag="phi0", name="u0")
                u1 = pool.tile([128, QC, S0], f32, tag="phi1", name="u1")
                u2 = pool.tile([128, QC, S0], f32, tag="phi2", name="u2")
                u3 = pool.tile([128, QC, S0], f32, tag="abs0", name="u3")
                u4 = pool.tile([128, QC, S0], f32, tag="abs1", name="u4")
                u5 = pool.tile([128, QC, S0], f32, tag="abs2", name="u5")
                u6 = pool.tile([128, QC, S0], f32, tag="sc0", name="u6")
                u7 = pool.tile([128, QC, S0], f32, tag="sc1", name="u7")
                u8 = pool.tile([128, QC, S0], f32, tag="sc2", name="u8")
                ubufs = [(u0, u1, u2), (u3, u4, u5), (u6, u7, u8)]

                def apply_unit(eng, i, ql, bufs, cc):
                    t0_, t1_, t2_ = bufs
                    eng.tensor_tensor(
                        t0_[:], W[:, ql, :, 0],
                        ShR[:, ql, i : i + 1].broadcast_to([128, QC, S0]), op=Al.mult
                    )
                    eng.tensor_tensor(
                        t1_[:], W[:, ql, :, 1],
                        ShR[:, ql, 3 + i : 4 + i].broadcast_to([128, QC, S0]), op=Al.mult
                    )
                    eng.tensor_tensor(
                        t2_[:], W[:, ql, :, 2],
                        ShR[:, ql, 6 + i : 7 + i].broadcast_to([128, QC, S0]), op=Al.mult
                    )
                    eng.tensor_tensor(t0_[:], t0_[:], t1_[:], op=Al.add)
                    eng.tensor_tensor(t0_[:], t0_[:], t2_[:], op=Al.add)
                    eng.tensor_tensor(
                        cc[i][:], t0_[:],
                        ShT[:, ql, i : i + 1].broadcast_to([128, QC, S0]), op=Al.add,
                    )

                with tc.tile_pool(name="psum", bufs=4, space="PSUM") as psum:
                    for qc in range(NCH):
                        ql = slice(qc * QC, (qc + 1) * QC)
                        # per-chunk double-buffered coords + Bk so the next
                        # chunk's compute overlaps this chunk's transposes/DMA
                        cc = [
                            pool.tile([128, QC, S0], f32, tag=f"coord{i}",
                                      name=f"coord{i}_{qc}", bufs=2)
                            for i in range(3)
                        ]
                        Bkc = pool.tile([128, NB2, 6, BC, 3], f32, tag="Bk",
                                        name=f"Bk{qc}", bufs=2)
                        apply_unit(nc.gpsimd, 2, ql, ubufs[2], cc)
                        apply_unit(nc.vector, 0, ql, ubufs[0], cc)
                        apply_unit(nc.vector, 1, ql, ubufs[1], cc)
                        for j in range(6):
                            for i in range(3):
                                cv = cc[i][:].rearrange("p q s -> p (q s)")
                                pt = psum.tile([128, 128], f32, tag="pt", name="pt")
                                nc.tensor.transpose(pt[:], cv[:, j * 128 : (j + 1) * 128], ident[:])
                                srcv = pt[:].rearrange("p (h c) -> p h c", h=2)
                                nc.scalar.copy(Bkc[:, :, j, :, i], srcv)
                        for h in range(NB2):
                            lo = h * NKB + qc * 6
                            nc.sync.dma_start(
                                out_dv[:, lo : lo + 6, :, :], Bkc[:, h]
                            )

            def dummy_out():
                nc.sync.dma_start(out_d[0:1, :, :], S["W"][0:1, 0:BC, 0, :])

            if only == "":
                for _rep in range(reps):
                    ph_l0(); ph_scan(); ph_apply()
            elif only == "l0":
                for _rep in range(reps):
                    ph_l0()
                dummy_out()
            elif only == "scan":
                ph_l0()
                for _rep in range(reps):
                    ph_scan()
                dummy_out()
            elif only == "apply":
                ph_l0(); ph_scan()
                for _rep in range(reps):
                    ph_apply()
            else:
                raise ValueError(only)

    nc.compile()
    return nc


def _get_program(reps: int = 1):
    import os as _os
    only = _os.environ.get("KM_ONLY", "")
    key = ("nc", reps, only)
    if key not in _CACHE:
        _CACHE[key] = _build_program(reps, only)
    return _CACHE[key]


def _make_mtab(core: int) -> np.ndarray:
    p = np.arange(128)
    c = p % 64
    bprime = 64 * core + c
    s = np.arange(S0)
    d = (2 * s[None, :] + bprime[:, None]) % 3
    mt = np.stack([_CT[d], _ST[d], np.broadcast_to(_BL[d], d.shape), _RST[d]], axis=-1)
    return np.ascontiguousarray(mt.astype(np.float32))


LAST_RUN = {}


def _get_runner(reps: int = 1):
    """Build (once) a cached jitted 8-core executable: inputs
    (phi_concat [8*K, BC], mtab_concat [8*128, S0, 4], out_zeros) -> out."""
    rkey = ("runner", reps)
    if rkey in _CACHE:
        return _CACHE[rkey]
    import jax
    import numpy as jnp_np  # noqa
    from jax.sharding import Mesh, PartitionSpec
    from jax.experimental.shard_map import shard_map
    from concourse import bass2jax, mybir

    nc = _get_program(reps)
    bass2jax.install_neuronx_cc_hook()

    partition_name = nc.partition_id_tensor.name if nc.partition_id_tensor else None
    in_names, out_names, out_avals = [], [], []
    for alloc in nc.m.functions[0].allocations:
        if not isinstance(alloc, mybir.MemoryLocationSet):
            continue
        name = alloc.memorylocations[0].name
        if alloc.kind == "ExternalInput":
            if name != partition_name:
                in_names.append(name)
        elif alloc.kind == "ExternalOutput":
            out_names.append(name)
            out_avals.append(
                jax.core.ShapedArray(tuple(alloc.tensor_shape), mybir.dt.np(alloc.dtype))
            )
    n_params = len(in_names)
    all_names = list(in_names) + list(out_names)
    if partition_name is not None:
        all_names.append(partition_name)

    def _body(*args):
        operands = list(args)
        if partition_name is not None:
            operands.append(bass2jax.partition_id_tensor())
        outs = bass2jax._bass_exec_p.bind(
            *operands,
            out_avals=tuple(out_avals),
            in_names=tuple(all_names),
            out_names=tuple(out_names),
            lowering_input_output_aliases=(),
            sim_require_finite=True,
            sim_require_nnan=True,
            nc=nc,
        )
        return tuple(outs)

    devices = jax.devices()[:NUM_CORES]
    mesh = Mesh(np.asarray(devices), ("core",))
    nin = n_params + len(out_names)
    sharded = jax.jit(
        shard_map(
            _body,
            mesh=mesh,
            in_specs=(PartitionSpec("core"),) * nin,
            out_specs=(PartitionSpec("core"),) * len(out_names),
            check_rep=False,
        ),
        keep_unused=True,
    )
    _CACHE[rkey] = (sharded, in_names, out_names, mesh)
    return _CACHE[rkey]


def _prep_inputs(phi: np.ndarray):
    """phi [K, B_FULL] -> dict of concatenated per-core inputs (device layouts)."""
    cores = []
    for c in range(NUM_CORES):
        pc = phi[:, c * BC : (c + 1) * BC]                  # [K, BC]
        # device layout [cch, p = 64*nb2 + c, q, sc]: k = 3072*nb2 + 24*q + 8*cch + sc
        arr = pc.reshape(NB2, NQ, 3, S0 // 3, BC).transpose(2, 0, 4, 1, 3)
        cores.append(np.ascontiguousarray(arr.reshape(3, 128, NQ, S0 // 3)))
    phi_cat = np.concatenate(cores, axis=0)
    mtab_cat = np.concatenate([_make_mtab(c) for c in range(NUM_CORES)], axis=0)
    return {"phi": phi_cat, "mtab": mtab_cat}


def _run(inputs: dict) -> np.ndarray:
    sharded, in_names, out_names, _ = _get_runner()
    zeros = [np.zeros((NUM_CORES * K, BC, 3), np.float32)]
    out_arrs = sharded(*[inputs[n] for n in in_names], *zeros)
    out_cat = np.asarray(out_arrs[0])
    return out_cat


def kernel(flat_dihedrals: np.ndarray, batch_size) -> np.ndarray:
    B = int(batch_size)
    assert B == B_FULL and flat_dihedrals.shape == (K * B // 3, 3), (
        f"hardcoded for {(K * B_FULL // 3, 3)}, got {flat_dihedrals.shape}, B={B}"
    )
    phi = np.ascontiguousarray(np.asarray(flat_dihedrals, dtype=np.float32).reshape(K, B))
    out_cat = _run(_prep_inputs(phi))
    per_core = out_cat.reshape(NUM_CORES, K, BC, 3)
    out = np.empty((K, B, 3), dtype=np.float32)
    for core in range(NUM_CORES):
        out[:, core * BC : (core + 1) * BC, :] = per_core[core]
    return out



# revision 11
# speedup vs baseline: 1.0228x; 1.0228x over previous
"""Trainium2 Bass kernel for the NeRF coordinate-chain problem.

Reference semantics: flat_dihedrals [1048576, 3] is (row-major) reinterpreted
as phi[K=6144, B=512]; each of the 512 columns is an independent serial NeRF
chain of K rigid-body extension steps, with bond-geometry constants cycling as
d = (q*B + b) mod 3.

Key reformulation: the per-step update is an affine (SE3) composition
    T_q = T_{q-1} o A_q,   A_q = [[G(phi,theta_d), p],[0,1]],
    G = Rx(phi) @ Rz(theta_d),  p = r_d * G[:,0],   T_0 = Identity
and coord_q = translation(T_q).  Associativity turns the 6144-step serial
recurrence into a blocked parallel scan:
  L0 upsweep: 256 blocks of S0=24 steps, vectorized over (block x batch)
    chains (structured Rx/Rz composes; w = r_d * c0_new);
  block-prefix: R-only Brent-Kung scan over the 128 blocks of each
    partition-half + translations via batched matvec and the hardware
    tensor_tensor_scan cumsum; cross-half fixup via a tiny SBUF DMA;
  apply: coords = ShR @ W + ShT per atom, chunked and pipelined with
    PE-transposes into k-major layout and contiguous output DMAs
    (GpSimd carries the contiguous adds).

Sharding: batch columns are split across 8 cores (64 columns/core); the
per-core chain layout uses partitions p = c + 64*nb2, free dims (q, s) with
block id nb = 128*nb2 + q.
"""

import numpy as np

L_STEPS = 2048
B_FULL = 512
NUM_CORES = 8
BC = B_FULL // NUM_CORES          # batch columns per core
K = 3 * L_STEPS                   # chain length = 6144
S0 = 24                           # L0 block size (multiple of 3)
NQ = 128                          # blocks per partition-half
NB2 = 2                           # partition halves (nb = 128*nb2 + q)
NB0 = NQ * NB2                    # 256 L0 blocks

_BL = np.array([145.801, 152.326, 132.868], dtype=np.float32)
_BA = np.array([2.124, 1.941, 2.028], dtype=np.float32)
_CT = np.cos(np.pi - _BA).astype(np.float32)
_ST = np.sin(np.pi - _BA).astype(np.float32)
_RCT = (_BL * _CT).astype(np.float32)
_RST = (_BL * _ST).astype(np.float32)

_CACHE = {}


def _build_program(reps: int = 1, only: str = ""):
    """Build the program.  `only` repeats a single phase inside the reps loop
    ("l0" | "scan" | "apply") for phase-level HW timing; "" = full kernel
    repeated per rep."""
    import concourse.bass as bass
    import concourse.tile as tile
    from concourse import bacc, masks, mybir
    from concourse._compat import axon_active

    f32 = mybir.dt.float32
    Al = mybir.AluOpType
    Act = mybir.ActivationFunctionType

    nc = bacc.Bacc(
        "TRN2",
        target_bir_lowering=False,
        debug=not axon_active(),
        enable_asserts=False,
        num_devices=NUM_CORES,
    )
    phi_d = nc.dram_tensor("phi", [3, 128, NQ, S0 // 3], f32, kind="ExternalInput").ap()
    mtab_d = nc.dram_tensor("mtab", [128, S0, 4], f32, kind="ExternalInput").ap()
    out_d = nc.dram_tensor("out", [K, BC, 3], f32, kind="ExternalOutput").ap()

    with tile.TileContext(nc) as tc:
        with tc.tile_pool(name="main", bufs=1) as pool:
            S = {}

            def ph_l0():
                # ---------------- load inputs + trig ----------------
                mtab = S["mtab"] = pool.tile([128, S0, 4], f32, tag="mtab", name="mtab")
                nc.sync.dma_start(mtab[:], mtab_d[:])
                SC = S0 // 3  # s-chunk size
                pih = pool.tile([128, 1], f32, tag="pih", name="pih")
                zero = pool.tile([128, 1], f32, tag="zero", name="zero")
                nc.vector.memset(pih[:], float(np.pi / 2))
                nc.vector.memset(zero[:], 0.0)
                # sc[..., 0]=cos(phi), 1=sin(phi), 2=-sin(phi); chunked over s
                scs = []
                for cch in range(3):
                    phi = pool.tile([128, NQ, SC], f32, tag=f"phi{cch}", name=f"phi{cch}")
                    nc.sync.dma_start(phi[:], phi_d[cch])
                    sct = pool.tile([128, NQ, SC, 3], f32, tag=f"sc{cch}", name=f"sc{cch}")
                    absphi = pool.tile([128, NQ, SC], f32, tag=f"abs{cch}", name=f"abs{cch}")
                    nc.scalar.activation(absphi[:], phi[:], Act.Abs, bias=zero[:, :])
                    nc.scalar.activation(sct[:, :, :, 1], phi[:], Act.Sin, bias=zero[:, :])
                    nc.scalar.activation(sct[:, :, :, 0], absphi[:], Act.Sin, bias=pih[:, :], scale=-1.0)
                    nc.vector.tensor_scalar(sct[:, :, :, 2], sct[:, :, :, 1], -1.0, None, op0=Al.mult)
                    scs.append(sct)

                # ---------------- L0 upsweep ----------------
                # R state [c0|c1|c2] column-major, per (p, q) chain
                R = S["R"] = pool.tile([128, NQ, 9], f32, tag="R", name="R")
                nc.vector.memset(R[:], 0.0)
                nc.vector.memset(R[:, :, 0:9:4], 1.0)
                W = S["W"] = pool.tile([128, NQ, S0, 3], f32, tag="W", name="W")
                m12 = pool.tile([128, NQ, 2, 3], f32, tag="m12", name="m12")
                m34 = pool.tile([128, NQ, 2, 3], f32, tag="m34", name="m34")
                tb0 = pool.tile([128, NQ, 3], f32, tag="tb0", name="tb0")
                tb1 = pool.tile([128, NQ, 3], f32, tag="tb1", name="tb1")

                R12 = R[:, :, 3:9].rearrange("p q (two three) -> p q two three", two=2)
                R21 = R12[:, :, ::-1, :]

                for s in range(S0):
                    sct = scs[s // SC]
                    cphB = sct[:, :, s % SC, 0:1].unsqueeze(2).broadcast_to([128, NQ, 2, 3])
                    snsB = sct[:, :, s % SC, 1:3].unsqueeze(3).broadcast_to([128, NQ, 2, 3])
                    # Rx(phi): (c1,c2) <- (cp*c1+sp*c2, cp*c2-sp*c1)
                    nc.vector.tensor_tensor(m12[:], R12, cphB, op=Al.mult)
                    nc.vector.tensor_tensor(m34[:], R21, snsB, op=Al.mult)
                    nc.vector.tensor_tensor(R12, m12[:], m34[:], op=Al.add)
                    # Rz(theta): (c0,c1) <- (ct*c0+st*c1, ct*c1-st*c0), fused stt
                    nc.vector.tensor_scalar(tb0[:], R[:, :, 3:6], mtab[:, s, 1:2], None, op0=Al.mult)
                    nc.vector.tensor_scalar(tb1[:], R[:, :, 0:3], mtab[:, s, 1:2], None, op0=Al.mult)
                    nc.vector.scalar_tensor_tensor(
                        R[:, :, 0:3], R[:, :, 0:3], mtab[:, s, 0:1], tb0[:], op0=Al.mult, op1=Al.add
                    )
                    nc.vector.scalar_tensor_tensor(
                        R[:, :, 3:6], R[:, :, 3:6], mtab[:, s, 0:1], tb1[:], op0=Al.mult, op1=Al.subtract
                    )
                    # w = R_old@p = r_d * c0_new ; W[s] = W[s-1] + w   (fused stt)
                    if s == 0:
                        nc.vector.tensor_scalar(W[:, :, 0, :], R[:, :, 0:3], mtab[:, s, 2:3], None, op0=Al.mult)
                    else:
                        nc.vector.scalar_tensor_tensor(
                            W[:, :, s, :], R[:, :, 0:3], mtab[:, s, 2:3], W[:, :, s - 1, :],
                            op0=Al.mult, op1=Al.add,
                        )

            def ph_scan():
                # ================= block-prefix phase =================
                R, W = S["R"], S["W"]
                ma = pool.tile([128, NQ, 9], f32, tag="sc0", name="ma")
                mb = pool.tile([128, NQ, 9], f32, tag="sc1", name="mb")
                mc = pool.tile([128, NQ, 9], f32, tag="sc2", name="mc")

                def compose_R(dst, a_R, b_R, P, g):
                    """dst[9] = Ra @ Rb (column-major cols)."""
                    pb = dst.base_partition()
                    for kk in range(3):
                        colk = a_R[:, :, 3 * kk : 3 * kk + 3].unsqueeze(2).broadcast_to([P, g, 3, 3])
                        rowk = b_R[:, :, kk::3].unsqueeze(3).broadcast_to([P, g, 3, 3])
                        dst_m = (ma, mb, mc)[kk]
                        mv = dst_m[pb : pb + P, 0:g].rearrange("p g (f t) -> p g f t", f=3)
                        nc.vector.tensor_tensor(mv, colk, rowk, op=Al.mult)
                    nc.vector.tensor_tensor(ma[pb : pb + P, 0:g], ma[pb : pb + P, 0:g], mb[pb : pb + P, 0:g], op=Al.add)
                    nc.vector.tensor_tensor(dst, ma[pb : pb + P, 0:g], mc[pb : pb + P, 0:g], op=Al.add)

                # Brent-Kung in-place inclusive scan over the q axis (per half)
                d = 1
                while d < NQ:
                    n = NQ // (2 * d)
                    a = R[:].rearrange("p (m j) t -> p m j t", j=2 * d)[:, :, d - 1, :]
                    b = R[:].rearrange("p (m j) t -> p m j t", j=2 * d)[:, :, 2 * d - 1, :]
                    compose_R(b, a, b, 128, n)
                    d *= 2
                d = NQ // 4
                while d >= 1:
                    n = NQ // (2 * d) - 1
                    a = R[:].rearrange("p (m j) t -> p m j t", j=2 * d)[:, 0:n, 2 * d - 1, :]
                    b = R[:].rearrange("p (m j) t -> p m j t", j=2 * d)[:, 1 : n + 1, d - 1, :]
                    compose_R(b, a, b, 128, n)
                    d //= 2
                PR = R  # inclusive R-prefix per half, in place

                # local shifted prefix (identity at q=0, both halves)
                ShR = S["ShR"] = pool.tile([128, NQ, 9], f32, tag="ShR", name="ShR")
                nc.vector.tensor_copy(ShR[:, 1:NQ, :], PR[:, 0 : NQ - 1, :])
                nc.vector.memset(ShR[:, 0, :], 0.0)
                nc.vector.memset(ShR[:, 0, 0:9:4], 1.0)

                # v[q] = ShR_local[q] @ t_loc[q] ; TI = cumsum_q(v) per half
                tloc = W[:, :, S0 - 1, :]
                v = pool.tile([128, NQ, 3], f32, tag="v", name="v")
                vz = pool.tile([128, NQ], f32, tag="vz", name="vz")
                nc.vector.memset(vz[:], 0.0)
                for kk in range(3):
                    colk = ShR[:, :, 3 * kk : 3 * kk + 3]
                    tk = tloc[:, :, kk : kk + 1].broadcast_to([128, NQ, 3])
                    dst_m = (ma, mb, mc)[kk]
                    nc.vector.tensor_tensor(dst_m[:, :, 0:3], colk, tk, op=Al.mult)
                nc.vector.tensor_tensor(ma[:, :, 0:3], ma[:, :, 0:3], mb[:, :, 0:3], op=Al.add)
                nc.vector.tensor_tensor(v[:], ma[:, :, 0:3], mc[:, :, 0:3], op=Al.add)
                TI = pool.tile([128, NQ, 3], f32, tag="TI", name="TI")
                for i in range(3):
                    nc.vector.tensor_tensor_scan(
                        TI[:, :, i], v[:, :, i], vz[:], 0.0, op0=Al.add, op1=Al.add
                    )
                # shifted translation prefix
                ShT = S["ShT"] = pool.tile([128, NQ, 3], f32, tag="ShT", name="ShT")
                nc.vector.tensor_copy(ShT[:, 1:NQ, :], TI[:, 0 : NQ - 1, :])
                nc.vector.memset(ShT[:, 0, :], 0.0)

                # cross-half: bring lower-half totals to upper partitions
                stgR = pool.tile([128, 1, 9], f32, tag="stgR", name="stgR")
                stgT = pool.tile([128, 1, 3], f32, tag="stgT", name="stgT")
                nc.sync.dma_start(stgR[64:128, :, :], PR[0:64, NQ - 1 : NQ, :])
                nc.sync.dma_start(stgT[64:128, :, :], TI[0:64, NQ - 1 : NQ, :])
                # ShR_up <- R_lowtot o ShR_up (in place)
                aR = stgR[64:128, :, :].broadcast_to([64, NQ, 9])
                compose_R(ShR[64:128, :, :], aR, ShR[64:128, :, :], 64, NQ)
                # ShT_up <- R_lowtot @ ShT_up + t_lowtot
                for kk in range(3):
                    colk = stgR[64:128, :, 3 * kk : 3 * kk + 3].broadcast_to([64, NQ, 3])
                    tk = ShT[64:128, :, kk : kk + 1].broadcast_to([64, NQ, 3])
                    dst_m = (ma, mb, mc)[kk]
                    nc.vector.tensor_tensor(dst_m[64:128, :, 0:3], colk, tk, op=Al.mult)
                nc.vector.tensor_tensor(ma[64:128, :, 0:3], ma[64:128, :, 0:3], mb[64:128, :, 0:3], op=Al.add)
                nc.vector.tensor_tensor(ShT[64:128, :, :], ma[64:128, :, 0:3], mc[64:128, :, 0:3], op=Al.add)
                nc.vector.tensor_tensor(
                    ShT[64:128, :, :], ShT[64:128, :, :],
                    stgT[64:128, :, :].broadcast_to([64, NQ, 3]), op=Al.add,
                )

            def ph_apply():
                # -------- L0 apply + transpose + store, pipelined by q-chunks --------
                # Each (chunk, i) unit runs entirely on one engine (2 on DVE, 1
                # on Pool) with disjoint scratch, so the engines proceed
                # concurrently with no per-step cross-engine ping-pong.
                W, ShR, ShT = S["W"], S["ShR"], S["ShT"]
                ident = pool.tile([128, 128], f32, tag="ident", name="ident")
                masks.make_identity(nc, ident[:])
                out_dv = out_d.rearrange("(kk p) c i -> p kk c i", p=128)
                NKB = 3072 // 128          # 24 kb tiles per half
                QC = 32                    # q-chunk; 32*24 = 768 = 6 kb tiles
                NCH = NQ // QC
                u0 = pool.tile([128, QC, S0], f32, tag="phi0", name="u0")
                u1 = pool.tile([128, QC, S0], f32, tag="phi1", name="u1")
                u2 = pool.tile([128, QC, S0], f32, tag="phi2", name="u2")
                u3 = pool.tile([128, QC, S0], f32, tag="abs0", name="u3")
                u4 = pool.tile([128, QC, S0], f32, tag="abs1", name="u4")
                u5 = pool.tile([128, QC, S0], f32, tag="abs2", name="u5")
                u6 = pool.tile([128, QC, S0], f32, tag="sc0", name="u6")
                u7 = pool.tile([128, QC, S0], f32, tag="sc1", name="u7")
                u8 = pool.tile([128, QC, S0], f32, tag="sc2", name="u8")
                ubufs = [(u0, u1, u2), (u3, u4, u5), (u6, u7, u8)]

                def apply_unit(eng, i, ql, bufs, cc):
                    t0_, t1_, t2_ = bufs
                    eng.tensor_tensor(
                        t0_[:], W[:, ql, :, 0],
                        ShR[:, ql, i : i + 1].broadcast_to([128, QC, S0]), op=Al.mult
                    )
                    eng.tensor_tensor(
                        t1_[:], W[:, ql, :, 1],
                        ShR[:, ql, 3 + i : 4 + i].broadcast_to([128, QC, S0]), op=Al.mult
                    )
                    eng.tensor_tensor(
                        t2_[:], W[:, ql, :, 2],
                        ShR[:, ql, 6 + i : 7 + i].broadcast_to([128, QC, S0]), op=Al.mult
                    )
                    eng.tensor_tensor(t0_[:], t0_[:], t1_[:], op=Al.add)
                    eng.tensor_tensor(t0_[:], t0_[:], t2_[:], op=Al.add)
                    eng.tensor_tensor(
                        cc[i][:], t0_[:],
                        ShT[:, ql, i : i + 1].broadcast_to([128, QC, S0]), op=Al.add,
                    )

                with tc.tile_pool(name="psum", bufs=4, space="PSUM") as psum:
                    for qc in range(NCH):
                        ql = slice(qc * QC, (qc + 1) * QC)
                        # per-chunk double-buffered coords + Bk so the next
                        # chunk's compute overlaps this chunk's transposes/DMA
                        cc = [
                            pool.tile([128, QC, S0], f32, tag=f"coord{i}",
                                      name=f"coord{i}_{qc}", bufs=2)
                            for i in range(3)
                        ]
                        Bkc = pool.tile([128, NB2, 6, BC, 3], f32, tag="Bk",
                                        name=f"Bk{qc}", bufs=2)
                        apply_unit(nc.gpsimd, 2, ql, ubufs[2], cc)
                        apply_unit(nc.vector, 0, ql, ubufs[0], cc)
                        apply_unit(nc.vector, 1, ql, ubufs[1], cc)
                        for j in range(6):
                            for i in range(3):
                                cv = cc[i][:].rearrange("p q s -> p (q s)")
                                pt = psum.tile([128, 128], f32, tag="pt", name="pt")
                                nc.tensor.transpose(pt[:], cv[:, j * 128 : (j + 1) * 128], ident[:])
                                srcv = pt[:].rearrange("p (h c) -> p h c", h=2)
                                nc.scalar.copy(Bkc[:, :, j, :, i], srcv)
                        for h in range(NB2):
                            lo = h * NKB + qc * 6
                            nc.sync.dma_start(
                                out_dv[:, lo : lo + 6, :, :], Bkc[:, h]
                            )

            def dummy_out():
                nc.sync.dma_start(out_d[0:1, :, :], S["W"][0:1, 0:BC, 0, :])

            if only == "":
                for _rep in range(reps):
                    ph_l0(); ph_scan(); ph_apply()
            elif only == "l0":
                for _rep in range(reps):
                    ph_l0()
                dummy_out()
            elif only == "scan":
                ph_l0()
                for _rep in range(reps):
                    ph_scan()
                dummy_out()
            elif only == "apply":
                ph_l0(); ph_scan()
                for _rep in range(reps):
                    ph_apply()
            else:
                raise ValueError(only)

    nc.compile()
    return nc


def _get_program(reps: int = 1):
    import os as _os
    only = _os.environ.get("KM_ONLY", "")
    key = ("nc", reps, only)
    if key not in _CACHE:
        _CACHE[key] = _build_program(reps, only)
    return _CACHE[key]


def _make_mtab(core: int) -> np.ndarray:
    p = np.arange(128)
    c = p % 64
    bprime = 64 * core + c
    s = np.arange(S0)
    d = (2 * s[None, :] + bprime[:, None]) % 3
    mt = np.stack([_CT[d], _ST[d], np.broadcast_to(_BL[d], d.shape), _RST[d]], axis=-1)
    return np.ascontiguousarray(mt.astype(np.float32))


LAST_RUN = {}


def _get_runner(reps: int = 1):
    """Build (once) a cached jitted 8-core executable: inputs
    (phi_concat [8*K, BC], mtab_concat [8*128, S0, 4], out_zeros) -> out."""
    rkey = ("runner", reps)
    if rkey in _CACHE:
        return _CACHE[rkey]
    import jax
    import numpy as jnp_np  # noqa
    from jax.sharding import Mesh, PartitionSpec
    from jax.experimental.shard_map import shard_map
    from concourse import bass2jax, mybir

    nc = _get_program(reps)
    bass2jax.install_neuronx_cc_hook()

    partition_name = nc.partition_id_tensor.name if nc.partition_id_tensor else None
    in_names, out_names, out_avals = [], [], []
    for alloc in nc.m.functions[0].allocations:
        if not isinstance(alloc, mybir.MemoryLocationSet):
            continue
        name = alloc.memorylocations[0].name
        if alloc.kind == "ExternalInput":
            if name != partition_name:
                in_names.append(name)
        elif alloc.kind == "ExternalOutput":
            out_names.append(name)
            out_avals.append(
                jax.core.ShapedArray(tuple(alloc.tensor_shape), mybir.dt.np(alloc.dtype))
            )
    n_params = len(in_names)
    all_names = list(in_names) + list(out_names)
    if partition_name is not None:
        all_names.append(partition_name)

    def _body(*args):
        operands = list(args)
        if partition_name is not None:
            operands.append(bass2jax.partition_id_tensor())
        outs = bass2jax._bass_exec_p.bind(
            *operands,
            out_avals=tuple(out_avals),
            in_names=tuple(all_names),
            out_names=tuple(out_names),
            lowering_input_output_aliases=(),
            sim_require_finite=True,
            sim_require_nnan=True,
            nc=nc,
        )
        return tuple(outs)

    devices = jax.devices()[:NUM_CORES]
    mesh = Mesh(np.asarray(devices), ("core",))
    nin = n_params + len(out_names)
    sharded = jax.jit(
        shard_map(
            _body,
            mesh=mesh,
            in_specs=(PartitionSpec("core"),) * nin,
            out_specs=(PartitionSpec("core"),) * len(out_names),
            check_rep=False,
        ),
        keep_unused=True,
    )
    _CACHE[rkey] = (sharded, in_names, out_names, mesh)
    return _CACHE[rkey]


def _prep_inputs(phi: np.ndarray):
    """phi [K, B_FULL] -> dict of concatenated per-core inputs (device layouts)."""
    cores = []
    for c in range(NUM_CORES):
        pc = phi[:, c * BC : (c + 1) * BC]                  # [K, BC]
        # device layout [cch, p = 64*nb2 + c, q, sc]: k = 3072*nb2 + 24*q + 8*cch + sc
        arr = pc.reshape(NB2, NQ, 3, S0 // 3, BC).transpose(2, 0, 4, 1, 3)
        cores.append(np.ascontiguousarray(arr.reshape(3, 128, NQ, S0 // 3)))
    phi_cat = np.concatenate(cores, axis=0)
    mtab_cat = np.concatenate([_make_mtab(c) for c in range(NUM_CORES)], axis=0)
    return {"phi": phi_cat, "mtab": mtab_cat}


def _run(inputs: dict) -> np.ndarray:
    sharded, in_names, out_names, _ = _get_runner()
    zeros = [np.zeros((NUM_CORES * K, BC, 3), np.float32)]
    out_arrs = sharded(*[inputs[n] for n in in_names], *zeros)
    out_cat = np.asarray(out_arrs[0])
    return out_cat


def kernel(flat_dihedrals: np.ndarray, batch_size) -> np.ndarray:
    B = int(batch_size)
    assert B == B_FULL and flat_dihedrals.shape == (K * B // 3, 3), (
        f"hardcoded for {(K * B_FULL // 3, 3)}, got {flat_dihedrals.shape}, B={B}"
    )
    phi = np.ascontiguousarray(np.asarray(flat_dihedrals, dtype=np.float32).reshape(K, B))
    out_cat = _run(_prep_inputs(phi))
    per_core = out_cat.reshape(NUM_CORES, K, BC, 3)
    out = np.empty((K, B, 3), dtype=np.float32)
    for core in range(NUM_CORES):
        out[:, core * BC : (core + 1) * BC, :] = per_core[core]
    return out



# revision 12
# speedup vs baseline: 2.7394x; 2.6784x over previous
"""Trainium2 Bass kernel for the NeRF coordinate-chain problem.

Reference semantics: flat_dihedrals [1048576, 3] is (row-major) reinterpreted
as phi[K=6144, B=512]; each of the 512 columns is an independent serial NeRF
chain of K rigid-body extension steps, with bond-geometry constants cycling as
d = (q*B + b) mod 3.

Key reformulation: the per-step update is an affine (SE3) composition
    T_q = T_{q-1} o A_q,   A_q = [[G(phi,theta_d), p],[0,1]],
    G = Rx(phi) @ Rz(theta_d),  p = r_d * G[:,0],   T_0 = Identity
and coord_q = translation(T_q).  Associativity turns the 6144-step serial
recurrence into a blocked parallel scan:
  L0 upsweep: 256 blocks of S0=24 steps, vectorized over (block x batch)
    chains (structured Rx/Rz composes; w = r_d * c0_new);
  block-prefix: R-only Brent-Kung scan over the 128 blocks of each
    partition-half + translations via batched matvec and the hardware
    tensor_tensor_scan cumsum; cross-half fixup via a tiny SBUF DMA;
  apply: coords = ShR @ W + ShT per atom, chunked and pipelined with
    PE-transposes into k-major layout and contiguous output DMAs
    (GpSimd carries the contiguous adds).

Sharding: batch columns are split across 8 cores (64 columns/core); the
per-core chain layout uses partitions p = c + 64*nb2, free dims (q, s) with
block id nb = 128*nb2 + q.
"""

import numpy as np

L_STEPS = 2048
B_FULL = 512
NUM_CORES = 8
BC = B_FULL // NUM_CORES          # batch columns per core
K = 3 * L_STEPS                   # chain length = 6144
S0 = 24                           # L0 block size (multiple of 3)
NQ = 128                          # blocks per partition-half
NB2 = 2                           # partition halves (nb = 128*nb2 + q)
NB0 = NQ * NB2                    # 256 L0 blocks

_BL = np.array([145.801, 152.326, 132.868], dtype=np.float32)
_BA = np.array([2.124, 1.941, 2.028], dtype=np.float32)
_CT = np.cos(np.pi - _BA).astype(np.float32)
_ST = np.sin(np.pi - _BA).astype(np.float32)
_RCT = (_BL * _CT).astype(np.float32)
_RST = (_BL * _ST).astype(np.float32)

_CACHE = {}


def _build_program(reps: int = 1, only: str = ""):
    """Build the program.  `only` repeats a single phase inside the reps loop
    ("l0" | "scan" | "apply") for phase-level HW timing; "" = full kernel
    repeated per rep."""
    import concourse.bass as bass
    import concourse.tile as tile
    from concourse import bacc, masks, mybir
    from concourse._compat import axon_active

    f32 = mybir.dt.float32
    Al = mybir.AluOpType
    Act = mybir.ActivationFunctionType

    nc = bacc.Bacc(
        "TRN2",
        target_bir_lowering=False,
        debug=not axon_active(),
        enable_asserts=False,
        num_devices=NUM_CORES,
    )
    phi_d = nc.dram_tensor("phi", [3, 128, NQ, S0 // 3], f32, kind="ExternalInput").ap()
    mtab_d = nc.dram_tensor("mtab", [128, S0, 4], f32, kind="ExternalInput").ap()
    out_d = nc.dram_tensor("out", [K, BC, 3], f32, kind="ExternalOutput").ap()

    with tile.TileContext(nc) as tc:
        with tc.tile_pool(name="main", bufs=1) as pool:
            S = {}

            def ph_l0():
                # ---------------- load inputs + trig ----------------
                mtab = S["mtab"] = pool.tile([128, S0, 4], f32, tag="mtab", name="mtab")
                nc.sync.dma_start(mtab[:], mtab_d[:])
                SC = S0 // 3  # s-chunk size
                pih = pool.tile([128, 1], f32, tag="pih", name="pih")
                zero = pool.tile([128, 1], f32, tag="zero", name="zero")
                nc.vector.memset(pih[:], float(np.pi / 2))
                nc.vector.memset(zero[:], 0.0)
                # sc[..., 0]=cos(phi), 1=sin(phi), 2=-sin(phi); chunked over s
                scs = []
                for cch in range(3):
                    phi = pool.tile([128, NQ, SC], f32, tag=f"phi{cch}", name=f"phi{cch}")
                    nc.sync.dma_start(phi[:], phi_d[cch])
                    sct = pool.tile([128, NQ, SC, 3], f32, tag=f"sc{cch}", name=f"sc{cch}")
                    absphi = pool.tile([128, NQ, SC], f32, tag=f"abs{cch}", name=f"abs{cch}")
                    nc.scalar.activation(absphi[:], phi[:], Act.Abs, bias=zero[:, :])
                    nc.scalar.activation(sct[:, :, :, 1], phi[:], Act.Sin, bias=zero[:, :])
                    nc.scalar.activation(sct[:, :, :, 0], absphi[:], Act.Sin, bias=pih[:, :], scale=-1.0)
                    nc.vector.tensor_scalar(sct[:, :, :, 2], sct[:, :, :, 1], -1.0, None, op0=Al.mult)
                    scs.append(sct)

                # ---------------- L0 upsweep ----------------
                # R state [c0|c1|c2] column-major, per (p, q) chain
                R = S["R"] = pool.tile([128, NQ, 9], f32, tag="R", name="R")
                nc.vector.memset(R[:], 0.0)
                nc.vector.memset(R[:, :, 0:9:4], 1.0)
                W = S["W"] = pool.tile([128, NQ, S0, 3], f32, tag="W", name="W")
                m12 = pool.tile([128, NQ, 2, 3], f32, tag="m12", name="m12")
                m34 = pool.tile([128, NQ, 2, 3], f32, tag="m34", name="m34")
                tb0 = pool.tile([128, NQ, 3], f32, tag="tb0", name="tb0")
                tb1 = pool.tile([128, NQ, 3], f32, tag="tb1", name="tb1")

                R12 = R[:, :, 3:9].rearrange("p q (two three) -> p q two three", two=2)
                R21 = R12[:, :, ::-1, :]

                for s in range(S0):
                    sct = scs[s // SC]
                    cphB = sct[:, :, s % SC, 0:1].unsqueeze(2).broadcast_to([128, NQ, 2, 3])
                    snsB = sct[:, :, s % SC, 1:3].unsqueeze(3).broadcast_to([128, NQ, 2, 3])
                    # Rx(phi): (c1,c2) <- (cp*c1+sp*c2, cp*c2-sp*c1)
                    nc.vector.tensor_tensor(m12[:], R12, cphB, op=Al.mult)
                    nc.vector.tensor_tensor(m34[:], R21, snsB, op=Al.mult)
                    nc.vector.tensor_tensor(R12, m12[:], m34[:], op=Al.add)
                    # Rz(theta): (c0,c1) <- (ct*c0+st*c1, ct*c1-st*c0), fused stt
                    nc.vector.tensor_scalar(tb0[:], R[:, :, 3:6], mtab[:, s, 1:2], None, op0=Al.mult)
                    nc.vector.tensor_scalar(tb1[:], R[:, :, 0:3], mtab[:, s, 1:2], None, op0=Al.mult)
                    nc.vector.scalar_tensor_tensor(
                        R[:, :, 0:3], R[:, :, 0:3], mtab[:, s, 0:1], tb0[:], op0=Al.mult, op1=Al.add
                    )
                    nc.vector.scalar_tensor_tensor(
                        R[:, :, 3:6], R[:, :, 3:6], mtab[:, s, 0:1], tb1[:], op0=Al.mult, op1=Al.subtract
                    )
                    # w = R_old@p = r_d * c0_new ; W[s] = W[s-1] + w   (fused stt)
                    if s == 0:
                        nc.vector.tensor_scalar(W[:, :, 0, :], R[:, :, 0:3], mtab[:, s, 2:3], None, op0=Al.mult)
                    else:
                        nc.vector.scalar_tensor_tensor(
                            W[:, :, s, :], R[:, :, 0:3], mtab[:, s, 2:3], W[:, :, s - 1, :],
                            op0=Al.mult, op1=Al.add,
                        )

            def ph_scan():
                # ================= block-prefix phase =================
                R, W = S["R"], S["W"]
                ma = pool.tile([128, NQ, 9], f32, tag="sc0", name="ma")
                mb = pool.tile([128, NQ, 9], f32, tag="sc1", name="mb")
                mc = pool.tile([128, NQ, 9], f32, tag="sc2", name="mc")

                def compose_R(dst, a_R, b_R, P, g):
                    """dst[9] = Ra @ Rb (column-major cols)."""
                    pb = dst.base_partition()
                    for kk in range(3):
                        colk = a_R[:, :, 3 * kk : 3 * kk + 3].unsqueeze(2).broadcast_to([P, g, 3, 3])
                        rowk = b_R[:, :, kk::3].unsqueeze(3).broadcast_to([P, g, 3, 3])
                        dst_m = (ma, mb, mc)[kk]
                        mv = dst_m[pb : pb + P, 0:g].rearrange("p g (f t) -> p g f t", f=3)
                        nc.vector.tensor_tensor(mv, colk, rowk, op=Al.mult)
                    nc.vector.tensor_tensor(ma[pb : pb + P, 0:g], ma[pb : pb + P, 0:g], mb[pb : pb + P, 0:g], op=Al.add)
                    nc.vector.tensor_tensor(dst, ma[pb : pb + P, 0:g], mc[pb : pb + P, 0:g], op=Al.add)

                # Brent-Kung in-place inclusive scan over the q axis (per half)
                d = 1
                while d < NQ:
                    n = NQ // (2 * d)
                    a = R[:].rearrange("p (m j) t -> p m j t", j=2 * d)[:, :, d - 1, :]
                    b = R[:].rearrange("p (m j) t -> p m j t", j=2 * d)[:, :, 2 * d - 1, :]
                    compose_R(b, a, b, 128, n)
                    d *= 2
                d = NQ // 4
                while d >= 1:
                    n = NQ // (2 * d) - 1
                    a = R[:].rearrange("p (m j) t -> p m j t", j=2 * d)[:, 0:n, 2 * d - 1, :]
                    b = R[:].rearrange("p (m j) t -> p m j t", j=2 * d)[:, 1 : n + 1, d - 1, :]
                    compose_R(b, a, b, 128, n)
                    d //= 2
                PR = R  # inclusive R-prefix per half, in place

                # local shifted prefix (identity at q=0, both halves)
                ShR = S["ShR"] = pool.tile([128, NQ, 9], f32, tag="ShR", name="ShR")
                nc.vector.tensor_copy(ShR[:, 1:NQ, :], PR[:, 0 : NQ - 1, :])
                nc.vector.memset(ShR[:, 0, :], 0.0)
                nc.vector.memset(ShR[:, 0, 0:9:4], 1.0)

                # v[q] = ShR_local[q] @ t_loc[q] ; TI = cumsum_q(v) per half
                tloc = W[:, :, S0 - 1, :]
                v = pool.tile([128, NQ, 3], f32, tag="v", name="v")
                vz = pool.tile([128, NQ], f32, tag="vz", name="vz")
                nc.vector.memset(vz[:], 0.0)
                for kk in range(3):
                    colk = ShR[:, :, 3 * kk : 3 * kk + 3]
                    tk = tloc[:, :, kk : kk + 1].broadcast_to([128, NQ, 3])
                    dst_m = (ma, mb, mc)[kk]
                    nc.vector.tensor_tensor(dst_m[:, :, 0:3], colk, tk, op=Al.mult)
                nc.vector.tensor_tensor(ma[:, :, 0:3], ma[:, :, 0:3], mb[:, :, 0:3], op=Al.add)
                nc.vector.tensor_tensor(v[:], ma[:, :, 0:3], mc[:, :, 0:3], op=Al.add)
                TI = pool.tile([128, NQ, 3], f32, tag="TI", name="TI")
                for i in range(3):
                    nc.vector.tensor_tensor_scan(
                        TI[:, :, i], v[:, :, i], vz[:], 0.0, op0=Al.add, op1=Al.add
                    )
                # shifted translation prefix
                ShT = S["ShT"] = pool.tile([128, NQ, 3], f32, tag="ShT", name="ShT")
                nc.vector.tensor_copy(ShT[:, 1:NQ, :], TI[:, 0 : NQ - 1, :])
                nc.vector.memset(ShT[:, 0, :], 0.0)

                # cross-half: bring lower-half totals to upper partitions
                stgR = pool.tile([128, 1, 9], f32, tag="stgR", name="stgR")
                stgT = pool.tile([128, 1, 3], f32, tag="stgT", name="stgT")
                nc.sync.dma_start(stgR[64:128, :, :], PR[0:64, NQ - 1 : NQ, :])
                nc.sync.dma_start(stgT[64:128, :, :], TI[0:64, NQ - 1 : NQ, :])
                # ShR_up <- R_lowtot o ShR_up (in place)
                aR = stgR[64:128, :, :].broadcast_to([64, NQ, 9])
                compose_R(ShR[64:128, :, :], aR, ShR[64:128, :, :], 64, NQ)
                # ShT_up <- R_lowtot @ ShT_up + t_lowtot
                for kk in range(3):
                    colk = stgR[64:128, :, 3 * kk : 3 * kk + 3].broadcast_to([64, NQ, 3])
                    tk = ShT[64:128, :, kk : kk + 1].broadcast_to([64, NQ, 3])
                    dst_m = (ma, mb, mc)[kk]
                    nc.vector.tensor_tensor(dst_m[64:128, :, 0:3], colk, tk, op=Al.mult)
                nc.vector.tensor_tensor(ma[64:128, :, 0:3], ma[64:128, :, 0:3], mb[64:128, :, 0:3], op=Al.add)
                nc.vector.tensor_tensor(ShT[64:128, :, :], ma[64:128, :, 0:3], mc[64:128, :, 0:3], op=Al.add)
                nc.vector.tensor_tensor(
                    ShT[64:128, :, :], ShT[64:128, :, :],
                    stgT[64:128, :, :].broadcast_to([64, NQ, 3]), op=Al.add,
                )

            def ph_apply():
                # -------- L0 apply + transpose + store, pipelined by q-chunks --------
                # Each (chunk, i) unit runs entirely on one engine (2 on DVE, 1
                # on Pool) with disjoint scratch, so the engines proceed
                # concurrently with no per-step cross-engine ping-pong.
                W, ShR, ShT = S["W"], S["ShR"], S["ShT"]
                ident = pool.tile([128, 128], f32, tag="ident", name="ident")
                masks.make_identity(nc, ident[:])
                out_dv = out_d.rearrange("(kk p) c i -> p kk c i", p=128)
                NKB = 3072 // 128          # 24 kb tiles per half
                QC = 32                    # q-chunk; 32*24 = 768 = 6 kb tiles
                NCH = NQ // QC
                u0 = pool.tile([128, QC, S0], f32, tag="phi0", name="u0")
                u1 = pool.tile([128, QC, S0], f32, tag="phi1", name="u1")
                u2 = pool.tile([128, QC, S0], f32, tag="phi2", name="u2")
                u3 = pool.tile([128, QC, S0], f32, tag="abs0", name="u3")
                u4 = pool.tile([128, QC, S0], f32, tag="abs1", name="u4")
                u5 = pool.tile([128, QC, S0], f32, tag="abs2", name="u5")
                u6 = pool.tile([128, QC, S0], f32, tag="sc0", name="u6")
                u7 = pool.tile([128, QC, S0], f32, tag="sc1", name="u7")
                u8 = pool.tile([128, QC, S0], f32, tag="sc2", name="u8")
                ubufs = [(u0, u1, u2), (u3, u4, u5), (u6, u7, u8)]

                def apply_unit(eng, i, ql, bufs, cc):
                    t0_, t1_, t2_ = bufs
                    eng.tensor_tensor(
                        t0_[:], W[:, ql, :, 0],
                        ShR[:, ql, i : i + 1].broadcast_to([128, QC, S0]), op=Al.mult
                    )
                    eng.tensor_tensor(
                        t1_[:], W[:, ql, :, 1],
                        ShR[:, ql, 3 + i : 4 + i].broadcast_to([128, QC, S0]), op=Al.mult
                    )
                    eng.tensor_tensor(
                        t2_[:], W[:, ql, :, 2],
                        ShR[:, ql, 6 + i : 7 + i].broadcast_to([128, QC, S0]), op=Al.mult
                    )
                    eng.tensor_tensor(t0_[:], t0_[:], t1_[:], op=Al.add)
                    eng.tensor_tensor(t0_[:], t0_[:], t2_[:], op=Al.add)
                    eng.tensor_tensor(
                        cc[i][:], t0_[:],
                        ShT[:, ql, i : i + 1].broadcast_to([128, QC, S0]), op=Al.add,
                    )

                with tc.tile_pool(name="psum", bufs=4, space="PSUM") as psum:
                    for qc in range(NCH):
                        ql = slice(qc * QC, (qc + 1) * QC)
                        # per-chunk double-buffered coords + Bk so the next
                        # chunk's compute overlaps this chunk's transposes/DMA
                        cc = [
                            pool.tile([128, QC, S0], f32, tag=f"coord{i}",
                                      name=f"coord{i}_{qc}", bufs=2)
                            for i in range(3)
                        ]
                        Bkc = pool.tile([128, NB2, 6, BC, 3], f32, tag="Bk",
                                        name=f"Bk{qc}", bufs=2)
                        apply_unit(nc.gpsimd, 2, ql, ubufs[2], cc)
                        apply_unit(nc.vector, 0, ql, ubufs[0], cc)
                        apply_unit(nc.vector, 1, ql, ubufs[1], cc)
                        for j in range(6):
                            for i in range(3):
                                cv = cc[i][:].rearrange("p q s -> p (q s)")
                                pt = psum.tile([128, 128], f32, tag="pt", name="pt")
                                nc.tensor.transpose(pt[:], cv[:, j * 128 : (j + 1) * 128], ident[:])
                                srcv = pt[:].rearrange("p (h c) -> p h c", h=2)
                                nc.scalar.copy(Bkc[:, :, j, :, i], srcv)
                        for h in range(NB2):
                            lo = h * NKB + qc * 6
                            nc.sync.dma_start(
                                out_dv[:, lo : lo + 6, :, :], Bkc[:, h]
                            )

            def dummy_out():
                nc.sync.dma_start(out_d[0:1, :, :], S["W"][0:1, 0:BC, 0, :])

            if only == "":
                for _rep in range(reps):
                    ph_l0(); ph_scan(); ph_apply()
            elif only == "l0":
                for _rep in range(reps):
                    ph_l0()
                dummy_out()
            elif only == "scan":
                ph_l0()
                for _rep in range(reps):
                    ph_scan()
                dummy_out()
            elif only == "l0scan":
                for _rep in range(reps):
                    ph_l0(); ph_scan()
                dummy_out()
            elif only == "apply":
                ph_l0(); ph_scan()
                for _rep in range(reps):
                    ph_apply()
            else:
                raise ValueError(only)

    nc.compile()
    return nc


def _get_program(reps: int = 1):
    import os as _os
    only = _os.environ.get("KM_ONLY", "")
    key = ("nc", reps, only)
    if key not in _CACHE:
        _CACHE[key] = _build_program(reps, only)
    return _CACHE[key]


def _make_mtab(core: int) -> np.ndarray:
    p = np.arange(128)
    c = p % 64
    bprime = 64 * core + c
    s = np.arange(S0)
    d = (2 * s[None, :] + bprime[:, None]) % 3
    mt = np.stack([_CT[d], _ST[d], np.broadcast_to(_BL[d], d.shape), _RST[d]], axis=-1)
    return np.ascontiguousarray(mt.astype(np.float32))


LAST_RUN = {}


def _get_runner(reps: int = 1):
    """Build (once) a cached jitted 8-core executable: inputs
    (phi_concat [8*K, BC], mtab_concat [8*128, S0, 4], out_zeros) -> out."""
    rkey = ("runner", reps)
    if rkey in _CACHE:
        return _CACHE[rkey]
    import jax
    import numpy as jnp_np  # noqa
    from jax.sharding import Mesh, PartitionSpec
    from jax.experimental.shard_map import shard_map
    from concourse import bass2jax, mybir

    nc = _get_program(reps)
    bass2jax.install_neuronx_cc_hook()

    partition_name = nc.partition_id_tensor.name if nc.partition_id_tensor else None
    in_names, out_names, out_avals = [], [], []
    for alloc in nc.m.functions[0].allocations:
        if not isinstance(alloc, mybir.MemoryLocationSet):
            continue
        name = alloc.memorylocations[0].name
        if alloc.kind == "ExternalInput":
            if name != partition_name:
                in_names.append(name)
        elif alloc.kind == "ExternalOutput":
            out_names.append(name)
            out_avals.append(
                jax.core.ShapedArray(tuple(alloc.tensor_shape), mybir.dt.np(alloc.dtype))
            )
    n_params = len(in_names)
    all_names = list(in_names) + list(out_names)
    if partition_name is not None:
        all_names.append(partition_name)

    def _body(*args):
        operands = list(args)
        if partition_name is not None:
            operands.append(bass2jax.partition_id_tensor())
        outs = bass2jax._bass_exec_p.bind(
            *operands,
            out_avals=tuple(out_avals),
            in_names=tuple(all_names),
            out_names=tuple(out_names),
            lowering_input_output_aliases=(),
            sim_require_finite=True,
            sim_require_nnan=True,
            nc=nc,
        )
        return tuple(outs)

    devices = jax.devices()[:NUM_CORES]
    mesh = Mesh(np.asarray(devices), ("core",))
    nin = n_params + len(out_names)
    sharded = jax.jit(
        shard_map(
            _body,
            mesh=mesh,
            in_specs=(PartitionSpec("core"),) * nin,
            out_specs=(PartitionSpec("core"),) * len(out_names),
            check_rep=False,
        ),
        keep_unused=True,
    )
    _CACHE[rkey] = (sharded, in_names, out_names, mesh)
    return _CACHE[rkey]


def _prep_inputs(phi: np.ndarray):
    """phi [K, B_FULL] -> dict of concatenated per-core inputs (device layouts)."""
    cores = []
    for c in range(NUM_CORES):
        pc = phi[:, c * BC : (c + 1) * BC]                  # [K, BC]
        # device layout [cch, p = 64*nb2 + c, q, sc]: k = 3072*nb2 + 24*q + 8*cch + sc
        arr = pc.reshape(NB2, NQ, 3, S0 // 3, BC).transpose(2, 0, 4, 1, 3)
        cores.append(np.ascontiguousarray(arr.reshape(3, 128, NQ, S0 // 3)))
    phi_cat = np.concatenate(cores, axis=0)
    mtab_cat = np.concatenate([_make_mtab(c) for c in range(NUM_CORES)], axis=0)
    return {"phi": phi_cat, "mtab": mtab_cat}


def _run(inputs: dict) -> np.ndarray:
    sharded, in_names, out_names, _ = _get_runner()
    zeros = [np.zeros((NUM_CORES * K, BC, 3), np.float32)]
    out_arrs = sharded(*[inputs[n] for n in in_names], *zeros)
    out_cat = np.asarray(out_arrs[0])
    return out_cat


def kernel(flat_dihedrals: np.ndarray, batch_size) -> np.ndarray:
    B = int(batch_size)
    assert B == B_FULL and flat_dihedrals.shape == (K * B // 3, 3), (
        f"hardcoded for {(K * B_FULL // 3, 3)}, got {flat_dihedrals.shape}, B={B}"
    )
    phi = np.ascontiguousarray(np.asarray(flat_dihedrals, dtype=np.float32).reshape(K, B))
    out_cat = _run(_prep_inputs(phi))
    per_core = out_cat.reshape(NUM_CORES, K, BC, 3)
    out = np.empty((K, B, 3), dtype=np.float32)
    for core in range(NUM_CORES):
        out[:, core * BC : (core + 1) * BC, :] = per_core[core]
    return out

